# Initial kernel scaffold
#
"""Cubemap mip-chain (repeated 2x2 avg-pool, 2048 -> 16) on 8 TRN2 NeuronCores.

Sharding: the W dimension of every face is split into 8 column strips of
256 px; core i owns strip i of all 6 faces. Pooling is local per 2x2 block,
so strips never exchange data.

Per-core kernel: for each face, load the [2048, 256, 3] strip as one SBUF
tile [128 partitions, 16 rows x 768 floats] (16 base rows per partition, so
the whole strip of a face is one 6 MB contiguous DMA). Levels 1-4 are then
pure free-dim pool_avg pairs (H-pool over adjacent rows inside a partition,
then W-pool over adjacent pixel pairs); each pool_avg divides by 2, so the
H+W pair gives the exact /4 of a 2x2 average. After level 4 each partition
holds exactly one L4 row; a small SBUF->SBUF DMA repacks 8 L4 rows per
partition (6 faces -> 96 partitions) and levels 5-7 repeat the same
free-dim pooling. Every HBM DMA is fully contiguous.
"""

import numpy as np

N_CORES = 8
FACES = 6
BASE_RES = 2048
C = 3
W_SHARD = BASE_RES // N_CORES  # 256
P = 128                        # SBUF partitions used for the main levels
R0 = BASE_RES // P             # 16 base rows per partition
J = 8                          # L4 rows per partition in the tail
TAIL_P = FACES * (P // J)      # 96 partitions for levels 5-7

_CACHE = {}


def _build_nc():
    from concourse import bass, mybir
    from concourse.tile import TileContext

    dt = mybir.dt.float32
    nc = bass.Bass()

    base = nc.declare_dram_parameter(
        "base", [FACES, BASE_RES, W_SHARD, C], dt, isOutput=False
    )
    outs = {
        lvl: nc.declare_dram_parameter(
            f"out{lvl}", [FACES, BASE_RES >> lvl, W_SHARD >> lvl, C], dt, isOutput=True
        )
        for lvl in range(1, 8)
    }

    with TileContext(nc) as tc:
        with (
            tc.tile_pool(name="l0", bufs=2) as pool_l0,
            tc.tile_pool(name="h", bufs=1) as pool_h,
            tc.tile_pool(name="lx", bufs=2) as pool_lx,
            tc.tile_pool(name="tail", bufs=1) as pool_tail,
        ):
            stage = pool_tail.tile([TAIL_P, J * (W_SHARD >> 4) * C], dt, tag="stage")

            for f in range(FACES):
                t0 = pool_l0.tile([P, R0 * W_SHARD * C], dt, tag="l0")
                nc.sync.dma_start(
                    out=t0[:, :],
                    in_=base[f].rearrange("(p r) w c -> p (r w c)", p=P),
                )

                cur, rows, wl = t0, R0, W_SHARD
                for lvl in range(1, 5):
                    rows //= 2
                    ht = pool_h.tile([P, rows * wl * C], dt, tag=f"h{lvl}")
                    nc.vector.pool_avg(
                        out=ht[:, :],
                        in_=cur[:, :].rearrange(
                            "p (h k x) -> p h x k", h=rows, k=2, x=wl * C
                        ),
                    )
                    wl //= 2
                    lt = pool_lx.tile([P, rows * wl * C], dt, tag=f"l{lvl}")
                    nc.vector.pool_avg(
                        out=lt[:, :],
                        in_=ht[:, :].rearrange(
                            "p (r w k c) -> p r w c k", r=rows, w=wl, k=2, c=C
                        ),
                    )
                    nc.scalar.dma_start(
                        out=outs[lvl][f].rearrange("(p r) w c -> p (r w c)", p=P),
                        in_=lt[:, :],
                    )
                    cur = lt

                # repack: 8 L4 rows (one per partition) -> 1 tail partition
                nc.scalar.dma_start(
                    out=stage[f * (P // J) : (f + 1) * (P // J), :].rearrange(
                        "q (j x) -> q j x", j=J
                    ),
                    in_=cur[:, :].rearrange("(q j) x -> q j x", q=P // J),
                )

            cur, rows, wl = stage, J, W_SHARD >> 4
            for lvl in range(5, 8):
                rows //= 2
                ht = pool_h.tile([TAIL_P, rows * wl * C], dt, tag=f"h{lvl}")
                nc.vector.pool_avg(
                    out=ht[:, :],
                    in_=cur[:, :].rearrange(
                        "p (h k x) -> p h x k", h=rows, k=2, x=wl * C
                    ),
                )
                wl //= 2
                lt = pool_lx.tile([TAIL_P, rows * wl * C], dt, tag=f"l{lvl}")
                nc.vector.pool_avg(
                    out=lt[:, :],
                    in_=ht[:, :].rearrange(
                        "p (r w k c) -> p r w c k", r=rows, w=wl, k=2, c=C
                    ),
                )
                nc.scalar.dma_start(
                    out=outs[lvl].rearrange(
                        "f (q r) w c -> (f q) (r w c)", q=P // J
                    ),
                    in_=lt[:, :],
                )
                cur = lt

    return nc


def get_nc():
    if "nc" not in _CACHE:
        _CACHE["nc"] = _build_nc()
    return _CACHE["nc"]


def shard_inputs(base: np.ndarray) -> list:
    return [
        {"base": np.ascontiguousarray(base[:, :, i * W_SHARD : (i + 1) * W_SHARD, :])}
        for i in range(N_CORES)
    ]


def gather_outputs(base: np.ndarray, results: list) -> tuple:
    mips = [base]
    for lvl in range(1, 8):
        mips.append(
            np.concatenate([r[f"out{lvl}"] for r in results], axis=2)
        )
    return tuple(mips)


def kernel(base: np.ndarray, **_ignored) -> tuple:
    from concourse.bass_utils import run_bass_kernel_spmd

    base = np.asarray(base, dtype=np.float32)
    nc = get_nc()
    in_maps = shard_inputs(base)
    res = run_bass_kernel_spmd(nc, in_maps, core_ids=list(range(N_CORES)))
    return gather_outputs(base, res.results)


# revision 7
# speedup vs baseline: 1.0113x; 1.0113x over previous
"""Cubemap mip-chain (repeated 2x2 avg-pool, 2048 -> 16) on 8 TRN2 NeuronCores.

Sharding: the W dimension of every face is split into 8 column strips of
256 px; core i owns strip i of all 6 faces. Pooling is local per 2x2 block,
so strips never exchange data.

Per-core kernel: each face's [2048, 256, 3] strip is loaded as two SBUF
half-tiles [128 partitions, 8 rows x 768 floats] (8 base rows per
partition; each partition's data is one fully contiguous 24 KB HBM chunk).
Levels 1-3 are pure free-dim pooling per half-tile: per level two DVE
tensor_add ops (adjacent-row pairs inside a partition, then adjacent-pixel
pairs), carrying UNSCALED sums (4^l x the true mip). Half h of partition p
ends level 3 holding exactly L3 row 2p+h, so level 4's H-pair is a
cross-tile tensor_add of the two halves in the same partition. The exact
4^-l scaling happens only on the store branch, as a ScalarE activation
Copy with fused scale — parallel to the DVE chain. After level 4 each
partition holds exactly one L4 row; a single SBUF->SBUF DMA per face
repacks 8 L4 rows per partition (6 faces -> 96 partitions) and levels 5-7
repeat the same scheme. Every HBM DMA is fully contiguous. Loads ride the
sync HWDGE ring; stores ride the scalar ring.
"""

import numpy as np

N_CORES = 8
FACES = 6
BASE_RES = 2048
C = 3
W_SHARD = BASE_RES // N_CORES  # 256
P = 128                        # SBUF partitions used for the main levels
R0 = BASE_RES // P             # 16 base rows per partition
HALF = R0 // 2                 # 8 rows per partition per half-tile
J = 8                          # L4 rows per partition in the tail
TAIL_P = FACES * (P // J)      # 96 partitions for levels 5-7

_CACHE = {}


def _build_nc():
    from concourse import bacc, mybir
    from concourse.tile import TileContext

    dt = mybir.dt.float32
    copy_fn = mybir.ActivationFunctionType.Copy
    nc = bacc.Bacc()

    base = nc.declare_dram_parameter(
        "base", [FACES, BASE_RES, W_SHARD, C], dt, isOutput=False
    )
    outs = {
        lvl: nc.declare_dram_parameter(
            f"out{lvl}", [FACES, BASE_RES >> lvl, W_SHARD >> lvl, C], dt, isOutput=True
        )
        for lvl in range(1, 8)
    }

    with TileContext(nc) as tc:
        with (
            tc.tile_pool(name="l0", bufs=4) as pool_l0,
            tc.tile_pool(name="h", bufs=2) as pool_h,
            tc.tile_pool(name="lx", bufs=2) as pool_lx,
            tc.tile_pool(name="l3", bufs=3) as pool_l3,
            tc.tile_pool(name="st", bufs=2) as pool_st,
            tc.tile_pool(name="tail", bufs=1) as pool_tail,
        ):
            stage = pool_tail.tile([TAIL_P, J * (W_SHARD >> 4) * C], dt, tag="stage")

            def hpool(cur, parts, rows, wl, tag, pool):
                """H-pair sum: rows -> rows//2, within partitions (unscaled)."""
                rows //= 2
                ht = pool.tile([parts, rows * wl * C], dt, tag=tag)
                av = cur[:, :].rearrange(
                    "p (h k x) -> p h k x", h=rows, k=2, x=wl * C
                )
                nc.vector.tensor_add(
                    out=ht[:, :].rearrange("p (h x) -> p h x", h=rows),
                    in0=av[:, :, 0, :],
                    in1=av[:, :, 1, :],
                )
                return ht, rows

            def wpool(ht, parts, rows, wl, tag, pool):
                """W-pair sum: wl -> wl//2, within partitions (unscaled)."""
                wl //= 2
                lt = pool.tile([parts, rows * wl * C], dt, tag=tag)
                wv = ht[:, :].rearrange(
                    "p (r w k c) -> p r w c k", r=rows, w=wl, k=2, c=C
                )
                nc.vector.tensor_add(
                    out=lt[:, :].rearrange("p (r w c) -> p r w c", r=rows, w=wl, c=C),
                    in0=wv[:, :, :, :, 0],
                    in1=wv[:, :, :, :, 1],
                )
                return lt, wl

            def store_scaled(lt, parts, lvl, out_ap):
                """Scale the unscaled level sum by 4^-lvl (ScalarE) and DMA out."""
                st = pool_st.tile([parts, lt.shape[1]], dt, tag=f"s{lvl}")
                nc.scalar.activation(
                    out=st[:, :], in_=lt[:, :], func=copy_fn, scale=0.25 ** lvl
                )
                nc.scalar.dma_start(out=out_ap, in_=st[:, :])

            for f in range(FACES):
                # base[f] viewed as [p, g, rows, w, c]: half g of partition p
                # holds base rows [16p + 8g, 16p + 8g + 8) — 24 KB contiguous.
                src = base[f].rearrange(
                    "(p g r) w c -> g p (r w c)", p=P, g=2, r=HALF
                )
                l3_halves = []
                for g in range(2):
                    t0 = pool_l0.tile([P, HALF * W_SHARD * C], dt, tag="l0")
                    nc.sync.dma_start(out=t0[:, :], in_=src[g])

                    cur, rows, wl = t0, HALF, W_SHARD
                    for lvl in range(1, 4):
                        ht, rows = hpool(cur, P, rows, wl, f"h{lvl}", pool_h)
                        dst_pool = pool_l3 if lvl == 3 else pool_lx
                        cur, wl = wpool(ht, P, rows, wl, f"l{lvl}", dst_pool)
                        # rows of mip lvl held by (p, g): 2^(3-lvl)*(2p+g)+j
                        n_lvl = BASE_RES >> lvl
                        jj = n_lvl // (2 * P)
                        view = outs[lvl][f].rearrange(
                            "(p g j) w c -> g p (j w c)", p=P, g=2, j=jj
                        )
                        store_scaled(cur, P, lvl, view[g])
                    l3_halves.append(cur)

                # level 4: H-pair = same partition across the two halves
                h4 = pool_h.tile([P, (W_SHARD >> 3) * C], dt, tag="h4")
                nc.vector.tensor_add(
                    out=h4[:, :], in0=l3_halves[0][:, :], in1=l3_halves[1][:, :]
                )
                l4, _ = wpool(h4, P, 1, W_SHARD >> 3, "l4", pool_lx)
                store_scaled(
                    l4, P, 4,
                    outs[4][f].rearrange("(p r) w c -> p (r w c)", p=P),
                )

                # repack: 8 L4 rows (one per partition) -> 1 tail partition.
                # The flat (partition-major) element orders of [128, 48] and
                # [16, 384] coincide, so this is a single straight DMA.
                nc.scalar.dma_start(
                    out=stage[f * (P // J) : (f + 1) * (P // J), :],
                    in_=l4[:, :],
                )

            cur, rows, wl = stage, J, W_SHARD >> 4
            for lvl in range(5, 8):
                ht, rows = hpool(cur, TAIL_P, rows, wl, f"h{lvl}", pool_h)
                cur, wl = wpool(ht, TAIL_P, rows, wl, f"l{lvl}", pool_lx)
                store_scaled(
                    cur, TAIL_P, lvl,
                    outs[lvl].rearrange("f (q r) w c -> (f q) (r w c)", q=P // J),
                )

    nc.compile()
    return nc


def get_nc():
    if "nc" not in _CACHE:
        _CACHE["nc"] = _build_nc()
    return _CACHE["nc"]


def shard_inputs(base: np.ndarray) -> list:
    return [
        {"base": np.ascontiguousarray(base[:, :, i * W_SHARD : (i + 1) * W_SHARD, :])}
        for i in range(N_CORES)
    ]


def gather_outputs(base: np.ndarray, results: list) -> tuple:
    mips = [base]
    for lvl in range(1, 8):
        mips.append(np.concatenate([r[f"out{lvl}"] for r in results], axis=2))
    return tuple(mips)


def kernel(base: np.ndarray, **_ignored) -> tuple:
    from concourse.bass_utils import run_bass_kernel_spmd

    base = np.asarray(base, dtype=np.float32)
    nc = get_nc()
    in_maps = shard_inputs(base)
    res = run_bass_kernel_spmd(nc, in_maps, core_ids=list(range(N_CORES)))
    return gather_outputs(base, res.results)


# revision 9
# speedup vs baseline: 1.0441x; 1.0325x over previous
"""Cubemap mip-chain (repeated 2x2 avg-pool, 2048 -> 16) on 8 TRN2 NeuronCores.

Sharding: the W dimension of every face is split into 8 column strips of
256 px; core i owns strip i of all 6 faces. Pooling is local per 2x2 block,
so strips never exchange data.

Per-core kernel: each face's [2048, 256, 3] strip is loaded as two SBUF
half-tiles [128 partitions, 8 rows x 768 floats] (8 base rows per
partition; each partition's data is one fully contiguous 24 KB HBM chunk).
Levels 1-3 are pure free-dim pooling per half-tile: per level two DVE
tensor_add ops (adjacent-row pairs inside a partition, then adjacent-pixel
pairs), carrying UNSCALED sums (4^l x the true mip). Half h of partition p
ends level 3 holding exactly L3 row 2p+h, so level 4's H-pair is a
cross-tile tensor_add of the two halves in the same partition. The exact
4^-l scaling happens only on the store branch, as a ScalarE activation
Copy with fused scale — parallel to the DVE chain. After level 4 each
partition holds exactly one L4 row; a single SBUF->SBUF DMA per face
repacks 8 L4 rows per partition (6 faces -> 96 partitions) and levels 5-7
repeat the same scheme. Every HBM DMA is fully contiguous. Loads ride the
sync HWDGE ring; stores ride the scalar ring.
"""

import numpy as np

N_CORES = 8
FACES = 6
BASE_RES = 2048
C = 3
W_SHARD = BASE_RES // N_CORES  # 256
P = 128                        # SBUF partitions used for the main levels
R0 = BASE_RES // P             # 16 base rows per partition
HALF = R0 // 2                 # 8 rows per partition per half-tile
J = 8                          # L4 rows per partition in the tail
TAIL_P = FACES * (P // J)      # 96 partitions for levels 5-7

_CACHE = {}


def _build_nc():
    from concourse import bacc, mybir
    from concourse.tile import TileContext

    dt = mybir.dt.float32
    copy_fn = mybir.ActivationFunctionType.Copy
    nc = bacc.Bacc()

    base = nc.declare_dram_parameter(
        "base", [FACES, BASE_RES, W_SHARD, C], dt, isOutput=False
    )
    outs = {
        lvl: nc.declare_dram_parameter(
            f"out{lvl}", [FACES, BASE_RES >> lvl, W_SHARD >> lvl, C], dt, isOutput=True
        )
        for lvl in range(1, 8)
    }

    with TileContext(nc) as tc:
        with (
            tc.tile_pool(name="l0", bufs=5) as pool_l0,
            tc.tile_pool(name="h", bufs=2) as pool_h,
            tc.tile_pool(name="lx", bufs=2) as pool_lx,
            tc.tile_pool(name="l3", bufs=3) as pool_l3,
            tc.tile_pool(name="st", bufs=2) as pool_st,
            tc.tile_pool(name="tail", bufs=1) as pool_tail,
        ):
            stage = pool_tail.tile([TAIL_P, J * (W_SHARD >> 4) * C], dt, tag="stage")

            def hpool(cur, parts, rows, wl, tag, pool):
                """H-pair sum: rows -> rows//2, within partitions (unscaled)."""
                rows //= 2
                ht = pool.tile([parts, rows * wl * C], dt, tag=tag)
                av = cur[:, :].rearrange(
                    "p (h k x) -> p h k x", h=rows, k=2, x=wl * C
                )
                nc.vector.tensor_add(
                    out=ht[:, :].rearrange("p (h x) -> p h x", h=rows),
                    in0=av[:, :, 0, :],
                    in1=av[:, :, 1, :],
                )
                return ht, rows

            def wpool(ht, parts, rows, wl, tag, pool):
                """W-pair sum: wl -> wl//2, within partitions (unscaled)."""
                wl //= 2
                lt = pool.tile([parts, rows * wl * C], dt, tag=tag)
                wv = ht[:, :].rearrange(
                    "p (r w k c) -> p r w c k", r=rows, w=wl, k=2, c=C
                )
                nc.vector.tensor_add(
                    out=lt[:, :].rearrange("p (r w c) -> p r w c", r=rows, w=wl, c=C),
                    in0=wv[:, :, :, :, 0],
                    in1=wv[:, :, :, :, 1],
                )
                return lt, wl

            def store_scaled(lt, parts, lvl, out_ap):
                """Scale the unscaled level sum by 4^-lvl (ScalarE) and DMA out."""
                st = pool_st.tile([parts, lt.shape[1]], dt, tag=f"s{lvl}")
                nc.scalar.activation(
                    out=st[:, :], in_=lt[:, :], func=copy_fn, scale=0.25 ** lvl
                )
                nc.scalar.dma_start(out=out_ap, in_=st[:, :])

            # Emit every load up-front so the sync sequencer's stream is pure
            # load dispatches — each gated only by its pool-slot release, never
            # stuck behind compute-dependent instructions.
            half_tiles = []
            for f in range(FACES):
                # base[f] viewed as [p, g, rows, w, c]: half g of partition p
                # holds base rows [16p + 8g, 16p + 8g + 8) — 24 KB contiguous.
                src = base[f].rearrange(
                    "(p g r) w c -> g p (r w c)", p=P, g=2, r=HALF
                )
                for g in range(2):
                    t0 = pool_l0.tile([P, HALF * W_SHARD * C], dt, tag="l0")
                    nc.sync.dma_start(out=t0[:, :], in_=src[g])
                    half_tiles.append(t0)

            for f in range(FACES):
                l3_halves = []
                for g in range(2):
                    cur, rows, wl = half_tiles[2 * f + g], HALF, W_SHARD
                    for lvl in range(1, 4):
                        ht, rows = hpool(cur, P, rows, wl, f"h{lvl}", pool_h)
                        dst_pool = pool_l3 if lvl == 3 else pool_lx
                        cur, wl = wpool(ht, P, rows, wl, f"l{lvl}", dst_pool)
                        # rows of mip lvl held by (p, g): 2^(3-lvl)*(2p+g)+j
                        n_lvl = BASE_RES >> lvl
                        jj = n_lvl // (2 * P)
                        view = outs[lvl][f].rearrange(
                            "(p g j) w c -> g p (j w c)", p=P, g=2, j=jj
                        )
                        store_scaled(cur, P, lvl, view[g])
                    l3_halves.append(cur)

                # level 4: H-pair = same partition across the two halves
                h4 = pool_h.tile([P, (W_SHARD >> 3) * C], dt, tag="h4")
                nc.vector.tensor_add(
                    out=h4[:, :], in0=l3_halves[0][:, :], in1=l3_halves[1][:, :]
                )
                l4, _ = wpool(h4, P, 1, W_SHARD >> 3, "l4", pool_lx)
                store_scaled(
                    l4, P, 4,
                    outs[4][f].rearrange("(p r) w c -> p (r w c)", p=P),
                )

                # repack: 8 L4 rows (one per partition) -> 1 tail partition.
                # The flat (partition-major) element orders of [128, 48] and
                # [16, 384] coincide, so this is a single straight DMA.
                nc.scalar.dma_start(
                    out=stage[f * (P // J) : (f + 1) * (P // J), :],
                    in_=l4[:, :],
                )

            cur, rows, wl = stage, J, W_SHARD >> 4
            for lvl in range(5, 8):
                ht, rows = hpool(cur, TAIL_P, rows, wl, f"h{lvl}", pool_h)
                cur, wl = wpool(ht, TAIL_P, rows, wl, f"l{lvl}", pool_lx)
                store_scaled(
                    cur, TAIL_P, lvl,
                    outs[lvl].rearrange("f (q r) w c -> (f q) (r w c)", q=P // J),
                )

    nc.compile()
    return nc


def get_nc():
    if "nc" not in _CACHE:
        _CACHE["nc"] = _build_nc()
    return _CACHE["nc"]


def shard_inputs(base: np.ndarray) -> list:
    return [
        {"base": np.ascontiguousarray(base[:, :, i * W_SHARD : (i + 1) * W_SHARD, :])}
        for i in range(N_CORES)
    ]


def gather_outputs(base: np.ndarray, results: list) -> tuple:
    mips = [base]
    for lvl in range(1, 8):
        mips.append(np.concatenate([r[f"out{lvl}"] for r in results], axis=2))
    return tuple(mips)


def kernel(base: np.ndarray, **_ignored) -> tuple:
    from concourse.bass_utils import run_bass_kernel_spmd

    base = np.asarray(base, dtype=np.float32)
    nc = get_nc()
    in_maps = shard_inputs(base)
    res = run_bass_kernel_spmd(nc, in_maps, core_ids=list(range(N_CORES)))
    return gather_outputs(base, res.results)
